# revision 2
# baseline (speedup 1.0000x reference)
"""Trainium2 Bass kernel for dense-transformer attention block.

Computes, for x [N, d] and weight [M, d] (N=M=8192, d=1024, fp32):
    scores = x @ W^T / sqrt(d)        # [N, M]
    probs  = softmax(scores, axis=-1)
    out    = probs @ W                # [N, d]

Sharding: rows of x (N) split across 8 NeuronCores; W replicated.

Per-core device algorithm (all matmuls bf16 with fp32 PSUM accumulation):
  - mm1 computes scores TRANSPOSED: sT[m_tile, n_block] = W @ x^T so that
    the softmax matmul (mm2) can consume exp(sT) directly as the stationary
    operand with W in natural [M, d] layout for the moving operand.
  - softmax denominators come from an extra ones-column matmul (sum over M
    via PSUM accumulation), so no partition-axis reduction is ever needed.
  - max-subtraction is skipped: scores/sqrt(d) ~ N(0,1), |s|<8, exp is safe
    in fp32.
  - final out = (u @ W) * (1/l) with the reciprocal applied per row after
    mm2, avoiding a pass over the [n, M] probability matrix.

Host side does the layout prep (transpose + bf16 cast + row sharding) and
the gather/concat of per-core outputs.
"""

import os
from contextlib import ExitStack

import numpy as np
import ml_dtypes

import concourse.bass as bass
import concourse.mybir as mybir
import concourse.tile as tile
from concourse import bacc
from concourse.bass import ts, ds
from concourse.bass_utils import run_bass_kernel_spmd

# Problem shape (hardcoded per contract; spec nn_Model_39676907887569)
N_FULL = 8192
D = 1024
M = 8192
N_CORES = 8
N_LOC = N_FULL // N_CORES  # 1024 rows per core
SCALE = 1.0 / 32.0         # 1/sqrt(d)

BF16 = mybir.dt.bfloat16
F32 = mybir.dt.float32
NP_BF16 = ml_dtypes.bfloat16


def build_nc(n_loc=N_LOC, d=D, m=M, nb_rows=256, scale=SCALE):
    """Build the per-core Bass program (same NEFF for all cores)."""
    assert n_loc % nb_rows == 0 and nb_rows % 128 == 0
    assert d % 128 == 0 and m % 128 == 0
    d_tiles = d // 128
    m_tiles = m // 128
    n_blocks = n_loc // nb_rows
    n_chunks = nb_rows // 128
    # mm2 moving-operand chunks over the output feature dim (<=512 per PSUM bank)
    d_chunks = [(i, min(512, d - i)) for i in range(0, d, 512)]

    nc = bacc.Bacc(
        "TRN2",
        target_bir_lowering=False,
        debug=False,
        enable_asserts=False,
        num_devices=1,
    )

    xT_dram = nc.dram_tensor("xT", [d, n_loc], BF16, kind="ExternalInput").ap()
    wT_dram = nc.dram_tensor("wT", [d, m], BF16, kind="ExternalInput").ap()
    w_dram = nc.dram_tensor("w", [m, d], BF16, kind="ExternalInput").ap()
    out_dram = nc.dram_tensor("out", [n_loc, d], F32, kind="ExternalOutput").ap()

    # DRAM views with the 128-partition dim innermost-first for SBUF loads
    xT_v = xT_dram.rearrange("(a p) n -> p a n", p=128)   # [128, d_tiles, n_loc]
    wT_v = wT_dram.rearrange("(a p) m -> p a m", p=128)   # [128, d_tiles, m]

    with tile.TileContext(nc) as tc:
        with ExitStack() as ctx:
            singles = ctx.enter_context(tc.tile_pool(name="singles", bufs=1))
            w2_pool = ctx.enter_context(tc.tile_pool(name="w2", bufs=4))
            u_pool = ctx.enter_context(tc.tile_pool(name="u", bufs=3))
            o_pool = ctx.enter_context(tc.tile_pool(name="o", bufs=3))
            r_pool = ctx.enter_context(tc.tile_pool(name="r", bufs=4))
            s_psum = ctx.enter_context(tc.tile_pool(name="s_ps", bufs=2, space="PSUM"))
            acc_psum = ctx.enter_context(tc.tile_pool(name="acc", bufs=1, space="PSUM"))

            # Resident weights / activations
            wT_sb = singles.tile([128, d_tiles, m], BF16)
            xT_sb = singles.tile([128, d_tiles, n_loc], BF16)
            ones_sb = singles.tile([128, 1], BF16)
            nc.vector.memset(ones_sb, 1.0)

            # Load xT per n_block slice first (mm1 consumes in this order),
            # then wT per m_tile slice so early m_tiles arrive first.
            for nb in range(n_blocks):
                nc.sync.dma_start(
                    xT_sb[:, :, ds(nb * nb_rows, nb_rows)],
                    xT_v[:, :, ds(nb * nb_rows, nb_rows)],
                )
            for mt in range(m_tiles):
                nc.sync.dma_start(
                    wT_sb[:, :, ts(mt, 128)],
                    wT_v[:, :, ts(mt, 128)],
                )

            for nb in range(n_blocks):
                # Per-n_chunk PSUM accumulators, live across the whole m loop
                acc = []
                for nch in range(n_chunks):
                    chunks = [
                        acc_psum.tile([128, sz], F32, tag=f"acc_{nch}_{ci}",
                                      name=f"acc_{nch}_{ci}")
                        for ci, (_, sz) in enumerate(d_chunks)
                    ]
                    lacc = acc_psum.tile([128, 1], F32, tag=f"accl_{nch}",
                                         name=f"accl_{nch}")
                    acc.append((chunks, lacc))

                # Software pipeline: mm2 for m_tile t is issued after mm1 for
                # t+1 so the PE never waits on the ACT exp.
                pending = None  # (uT, w2) awaiting mm2

                def issue_mm2(uT, w2, first, last):
                    for nch in range(n_chunks):
                        lhsT = uT[:, ts(nch, 128)]
                        chunks, lacc = acc[nch]
                        for ci, (off, sz) in enumerate(d_chunks):
                            nc.tensor.matmul(
                                chunks[ci],
                                lhsT=lhsT,
                                rhs=w2[:, ds(off, sz)],
                                start=first,
                                stop=last,
                            )
                        nc.tensor.matmul(
                            lacc, lhsT=lhsT, rhs=ones_sb, start=first, stop=last
                        )

                for mt in range(m_tiles):
                    w2 = w2_pool.tile([128, d], BF16)
                    nc.sync.dma_start(w2, w_dram[ts(mt, 128), :])

                    s_ps = s_psum.tile([128, nb_rows], F32)
                    for dt_ in range(d_tiles):
                        nc.tensor.matmul(
                            s_ps,
                            lhsT=wT_sb[:, dt_, ts(mt, 128)],
                            rhs=xT_sb[:, dt_, ds(nb * nb_rows, nb_rows)],
                            start=(dt_ == 0),
                            stop=(dt_ == d_tiles - 1),
                        )
                    uT = u_pool.tile([128, nb_rows], BF16)
                    nc.scalar.activation(uT, s_ps, mybir.ActivationFunctionType.Exp,
                                         scale=scale)

                    if pending is not None:
                        issue_mm2(*pending, first=(mt == 1), last=False)
                    pending = (uT, w2)

                issue_mm2(*pending, first=(m_tiles == 1), last=True)

                # Normalize and store this n_block
                for nch in range(n_chunks):
                    chunks, lacc = acc[nch]
                    rcp = r_pool.tile([128, 1], F32)
                    nc.vector.reciprocal(rcp, lacc)
                    o = o_pool.tile([128, d], F32)
                    for ci, (off, sz) in enumerate(d_chunks):
                        nc.vector.tensor_scalar_mul(
                            o[:, ds(off, sz)], in0=chunks[ci], scalar1=rcp
                        )
                    row0 = nb * nb_rows + nch * 128
                    nc.sync.dma_start(out_dram[ds(row0, 128), :], o)

    nc.compile()
    return nc


_NC_CACHE = {}


def _get_nc(key=(N_LOC, D, M)):
    if key not in _NC_CACHE:
        _NC_CACHE[key] = build_nc(*key)
    return _NC_CACHE[key]


def kernel(x: np.ndarray, weight: np.ndarray) -> np.ndarray:
    x = np.ascontiguousarray(np.asarray(x, dtype=np.float32))
    w = np.ascontiguousarray(np.asarray(weight, dtype=np.float32))
    assert x.shape == (N_FULL, D) and w.shape == (M, D)

    # Host-side layout prep (cheap vs device work): bf16 casts + transposes
    w_bf = w.astype(NP_BF16)
    wT_bf = np.ascontiguousarray(w_bf.T)                      # [d, M]
    x_bf = x.astype(NP_BF16)
    xT_full = np.ascontiguousarray(x_bf.T)                    # [d, N]

    in_maps = []
    for c in range(N_CORES):
        xT_c = np.ascontiguousarray(xT_full[:, c * N_LOC:(c + 1) * N_LOC])
        in_maps.append({"xT": xT_c, "wT": wT_bf, "w": w_bf})

    nc = _get_nc()
    trace = bool(int(os.environ.get("KERNEL_TRACE", "0")))
    res = run_bass_kernel_spmd(
        nc,
        in_maps,
        core_ids=list(range(N_CORES)),
        trace=trace,
    )
    if trace and res.exec_time_ns is not None:
        print(f"HW exec time: {res.exec_time_ns} ns")
        kernel.last_results = res
    out = np.concatenate([r["out"] for r in res.results], axis=0)
    return out


kernel.last_results = None
